# revision 11
# baseline (speedup 1.0000x reference)
"""Distributed causal multi-head attention (RoPE) for 8 TRN2 NeuronCores.

Problem: B=4, S=2048, D=2048, H=16 heads, DH=128.
Sharding: 2D — data-parallel over the 4 batches x tensor-parallel over 2
head-groups of 8 heads (Megatron-style: Wqkv column-sharded per head
group, Wo row-sharded).  Core c handles batch c//2, head group c%2.
Each core returns a partial output projection [S, D]; the host sums the
two group partials per batch (the "all-reduce") and stacks batches.

Per-core pipeline (all matmuls on the PE array):
  stage 1: QKV projection from xT (d-major), RoPE fused into the
           PSUM eviction for Q/K (f32r), V evicted as bf16.
  stage 2: per head: scoresT = K^T-tiles x Q (f32r, N=512), exp via
           ACT (scale=1/sqrt(128)) into bf16 tiles, causal masking by
           multiplying precomputed mask tiles, PV via bf16 matmuls with
           a fused ones-column (denominator), per-q-tile normalization +
           PE transpose into resident zT tiles.
  stage 3: output projection out = sum_h zT_h.T @ WoT_h (f32r, N=512).
"""

import sys

if '/opt/trn_rl_repo' not in sys.path:
    sys.path.insert(0, '/opt/trn_rl_repo')

import math

import ml_dtypes
import numpy as np

B, S, D, H, DH = 4, 2048, 2048, 16, 128
BASE = 10000.0
P = 128
NT = S // P          # 16 token tiles of 128
NC512 = S // 512     # 4 token chunks of 512
NDM = D // P         # 16 d_model chunks
HG = 8               # heads per group
SCALE = 1.0 / math.sqrt(DH)

_CACHE = {}


def _build_program():
    import concourse.bacc as bacc
    import concourse.mybir as mybir
    from concourse.tile import TileContext
    from concourse.masks import make_identity

    F32 = mybir.dt.float32
    F32R = mybir.dt.float32r
    BF16 = mybir.dt.bfloat16
    EXP = mybir.ActivationFunctionType.Exp

    nc = bacc.Bacc('TRN2', target_bir_lowering=False, debug=False, num_devices=8)

    # ---- DRAM I/O ----
    xT = nc.dram_tensor('xT', [P, NDM, S], F32R, kind='ExternalInput').ap()
    wqkT = nc.dram_tensor('wqkT', [2 * HG, P, NDM, P], F32R, kind='ExternalInput').ap()
    wvT = nc.dram_tensor('wvT', [P, NDM, HG * P], F32R, kind='ExternalInput').ap()
    woT = nc.dram_tensor('woT', [P, HG, D], F32R, kind='ExternalInput').ap()
    cosT = nc.dram_tensor('cosT', [P, S], F32, kind='ExternalInput').ap()
    sinP = nc.dram_tensor('sinP', [P, S], F32, kind='ExternalInput').ap()
    maskT = nc.dram_tensor('maskT', [P, 4, 512], BF16, kind='ExternalInput').ap()
    onesb = nc.dram_tensor('onesb', [P, 1], BF16, kind='ExternalInput').ap()
    out = nc.dram_tensor('out', [NT, P, D], F32, kind='ExternalOutput').ap()

    # ---- DRAM scratch ----
    qk_scr = nc.dram_tensor('qk_scr', [2 * HG, P, S], F32R).ap()
    v_scr = nc.dram_tensor('v_scr', [NT, P, HG * P], BF16).ap()

    with TileContext(nc) as tc:
        # ================= stage 1: QKV projection =================
        with tc.tile_pool(name='s1x', bufs=1) as xpool:
            xsb = xpool.tile([P, NDM, S], F32R)
            for o in range(NDM):
                nc.sync.dma_start(xsb[:, o, 0:S // 2], xT[:, o, 0:S // 2])
                nc.sync.dma_start(xsb[:, o, S // 2:S], xT[:, o, S // 2:S])
            cos_sb = xpool.tile([P, S], F32)
            sin_sb = xpool.tile([P, S], F32)
            nc.sync.dma_start(cos_sb[:], cosT[:])
            nc.sync.dma_start(sin_sb[:], sinP[:])

            # --- Q and K blocks, feature-major (d_head on partitions) ---
            with tc.tile_pool(name='s1w', bufs=2) as wpool, \
                 tc.tile_pool(name='s1e', bufs=3) as epool, \
                 tc.tile_pool(name='s1p', bufs=4, space='PSUM') as ppool:
                for fb in range(2 * HG):
                    wsb = wpool.tile([P, NDM, P], F32R, tag='wqk', name=f'wqk{fb}')
                    nc.sync.dma_start(wsb[:], wqkT[fb])
                    for tcn in range(NC512):
                        ts = slice(tcn * 512, tcn * 512 + 512)
                        ps = ppool.tile([P, 512], F32, tag='pqk', name=f'pqk_{fb}_{tcn}')
                        for o in range(NDM):
                            nc.tensor.matmul(ps[:], wsb[:, o, :], xsb[:, o, ts],
                                             start=(o == 0), stop=(o == NDM - 1))
                        # RoPE fused eviction
                        t1 = epool.tile([P, 512], F32, tag='t1', name=f't1_{fb}_{tcn}')
                        t2 = epool.tile([P, 512], F32, tag='t2', name=f't2_{fb}_{tcn}')
                        qt = epool.tile([P, 512], F32R, tag='qt', name=f'qt_{fb}_{tcn}')
                        nc.vector.tensor_mul(t1[:], ps[:], cos_sb[:, ts])
                        # rotate_half via cross-partition reads (sign folded in sinP)
                        nc.vector.tensor_mul(t2[0:64, :], ps[64:128, :], sin_sb[0:64, ts])
                        nc.vector.tensor_mul(t2[64:128, :], ps[0:64, :], sin_sb[64:128, ts])
                        nc.vector.tensor_add(qt[:], t1[:], t2[:])
                        nc.sync.dma_start(qk_scr[fb][:, ts], qt[:])

            # --- V blocks, token-major ---
            with tc.tile_pool(name='s1wv', bufs=2) as wvpool, \
                 tc.tile_pool(name='s1ev', bufs=3) as evpool, \
                 tc.tile_pool(name='s1pv', bufs=4, space='PSUM') as pvpool:
                for vc in range(4):
                    vs = slice(vc * 256, vc * 256 + 256)
                    wv = wvpool.tile([P, NDM, 256], F32R, tag='wv', name=f'wv{vc}')
                    for o in range(NDM):
                        nc.sync.dma_start(wv[:, o, :], wvT[:, o, vs])
                    for tt in range(NT):
                        psv = pvpool.tile([P, 256], F32, tag='pv', name=f'pv_{vc}_{tt}')
                        for o in range(NDM):
                            nc.tensor.matmul(psv[:],
                                             xsb[:, o, tt * P:(tt + 1) * P],
                                             wv[:, o, :],
                                             start=(o == 0), stop=(o == NDM - 1))
                        vsb = evpool.tile([P, 256], BF16, tag='vsb', name=f'vsb_{vc}_{tt}')
                        nc.scalar.copy(vsb[:], psv[:])
                        nc.sync.dma_start(v_scr[tt][:, vs], vsb[:])

        # ========= stage 2+3: attention with folded output projection =========
        # Heads run in 2 rounds of 4; each round's output projection partial
        # is written (round 0) / DMA-accumulated (round 1) into `out`, so the
        # round-1 attention ACT work overlaps round-0's projection matmuls.
        with tc.tile_pool(name='s2c', bufs=1) as cpool, \
             tc.tile_pool(name='s2zt', bufs=1) as ztpool, \
             tc.tile_pool(name='s2qk', bufs=2) as qkpool, \
             tc.tile_pool(name='s2va', bufs=1) as vapool, \
             tc.tile_pool(name='s2st', bufs=2) as stpool, \
             tc.tile_pool(name='s2z', bufs=3) as zpool, \
             tc.tile_pool(name='s2wo', bufs=1) as wopool, \
             tc.tile_pool(name='s2os', bufs=2) as ospool, \
             tc.tile_pool(name='s2p', bufs=2, space='PSUM') as sppool, \
             tc.tile_pool(name='s2pz', bufs=2, space='PSUM') as zppool, \
             tc.tile_pool(name='s2pt', bufs=2, space='PSUM') as tppool, \
             tc.tile_pool(name='s2po', bufs=2, space='PSUM') as oppool:
            msk = cpool.tile([P, 4, 512], BF16)
            nc.sync.dma_start(msk[:], maskT[:])
            ident = cpool.tile([P, P], F32)
            make_identity(nc, ident[:])
            ones_sb = cpool.tile([P, 1], BF16)
            nc.sync.dma_start(ones_sb[:], onesb[:])

            zT = [ztpool.tile([P, S], F32R, name=f'zT{h}') for h in range(HG)]

            def emit_proj_group(rnd, tt, ec):
                """One (token-tile, e-chunk) group of the round projection."""
                hs = list(range(4 * rnd, 4 * rnd + 4))
                es = slice(ec * 512, ec * 512 + 512)
                wo = _wo_tiles[(rnd, ec)]
                pso = oppool.tile([P, 512], F32, tag='pso',
                                  name=f'pso_{rnd}_{tt}_{ec}')
                for hi, h in enumerate(hs):
                    nc.tensor.matmul(pso[:], zT[h][:, tt * P:(tt + 1) * P],
                                     wo[:, hi, :],
                                     start=(hi == 0), stop=(hi == 3))
                osb = ospool.tile([P, 512], F32, tag='osb',
                                  name=f'osb_{rnd}_{tt}_{ec}')
                if rnd == 0:
                    if (tt + ec) % 2 == 0:
                        nc.scalar.copy(osb[:], pso[:])
                    else:
                        nc.vector.tensor_copy(osb[:], pso[:])
                else:
                    obo = ospool.tile([P, 512], F32, tag='obo',
                                      name=f'obo_{tt}_{ec}')
                    nc.sync.dma_start(obo[:], out[tt][:, es])
                    nc.vector.tensor_add(osb[:], pso[:], obo[:])
                nc.sync.dma_start(out[tt][:, es], osb[:])

            def emit_head(h, filler):
                """Attention for head h; `filler(n)` emits up to n projection
                groups of independent PE work between the dependency-chained
                blocks (fills exp-wait bubbles)."""
                qt_h = qkpool.tile([P, S], F32R, tag='qt_h', name=f'qt_h{h}')
                kt_h = qkpool.tile([P, S], F32R, tag='kt_h', name=f'kt_h{h}')
                nc.sync.dma_start(qt_h[:], qk_scr[h])
                nc.sync.dma_start(kt_h[:], qk_scr[HG + h])
                vau = []
                for kt in range(NT):
                    va = vapool.tile([P, P + 1], BF16, tag=f'vau{kt}',
                                     name=f'vau_{h}_{kt}')
                    nc.sync.dma_start(va[:, 0:P], v_scr[kt][:, h * P:(h + 1) * P])
                    nc.vector.tensor_copy(va[:, P:P + 1], ones_sb[:])
                    vau.append(va)

                st = [[None] * NT for _ in range(NC512)]

                def emit_qk(qr):
                    qs512 = slice(qr * 512, qr * 512 + 512)
                    for kt in range(4 * qr + 4):
                        sps = sppool.tile([P, 512], F32, tag='sps',
                                          name=f'sps_{h}_{qr}_{kt}')
                        nc.tensor.matmul(sps[:], kt_h[:, kt * P:(kt + 1) * P],
                                         qt_h[:, qs512], start=True, stop=True)
                        stt = stpool.tile([P, 512], BF16, tag=f'st{kt}',
                                          name=f'st_{h}_{qr}_{kt}')
                        nc.scalar.activation(stt[:], sps[:], EXP, scale=SCALE)
                        d = kt - 4 * qr
                        if d >= 0:
                            nc.vector.tensor_mul(stt[:], stt[:], msk[:, d, :])
                        st[qr][kt] = stt

                emit_qk(0)
                for qr in range(NC512):
                    if qr + 1 < NC512:
                        emit_qk(qr + 1)   # QK(qr+1) runs while ACT exps qr
                    filler(4)
                    for qs in range(4):
                        qa = 4 * qr + qs
                        zps = zppool.tile([P, P + 1], F32, tag='zps',
                                          name=f'zps_{h}_{qa}')
                        for kt in range(qa + 1):
                            nc.tensor.matmul(zps[:],
                                             st[qr][kt][:, qs * P:(qs + 1) * P],
                                             vau[kt][:],
                                             start=(kt == 0), stop=(kt == qa))
                        rcp = zpool.tile([P, 1], F32, tag='rcp',
                                         name=f'rcp_{h}_{qa}')
                        nc.vector.reciprocal(rcp[:], zps[:, P:P + 1])
                        zsb = zpool.tile([P, P], F32, tag='zsb',
                                         name=f'zsb_{h}_{qa}')
                        nc.scalar.activation(zsb[:], zps[:, 0:P],
                                             mybir.ActivationFunctionType.Copy,
                                             scale=rcp[:])
                        ztp = tppool.tile([P, P], F32, tag='ztp',
                                          name=f'ztp_{h}_{qa}')
                        nc.tensor.transpose(ztp[:], zsb[:], ident[:])
                        nc.vector.tensor_copy(zT[h][:, qa * P:(qa + 1) * P],
                                              ztp[:])

            # work queue of round-0 projection groups, drained as filler
            # inside round-1 heads (and any leftovers after)
            _wo_tiles = {}

            def load_wo(rnd):
                for ec in range(NC512):
                    wo = wopool.tile([P, 4, 512], F32R, tag=f'wo{ec}',
                                     name=f'wo_{rnd}_{ec}')
                    for hi, h in enumerate(range(4 * rnd, 4 * rnd + 4)):
                        nc.sync.dma_start(wo[:, hi, :], woT[:, h, ec * 512:(ec + 1) * 512])
                    _wo_tiles[(rnd, ec)] = wo

            pending = []

            def filler(n):
                for _ in range(min(n, len(pending))):
                    emit_proj_group(*pending.pop(0))

            for h in range(4):
                emit_head(h, lambda n: None)
            load_wo(0)
            pending = [(0, tt, ec) for ec in range(NC512) for tt in range(NT)]
            for h in range(4, 8):
                emit_head(h, filler)
            filler(len(pending))
            load_wo(1)
            pending = [(1, tt, ec) for ec in range(NC512) for tt in range(NT)]
            filler(len(pending))

    nc.compile()
    return nc


def _host_inputs(x, Wqkv, Wo):
    """Build the 8 per-core input maps."""
    # RoPE tables (match reference: float32 math)
    inv_freq = (1.0 / (BASE ** (np.arange(0, DH, 2, dtype=np.float32) / DH))).astype(np.float32)
    t = np.arange(S, dtype=np.float32)
    freqs = np.einsum('i,j->ij', t, inv_freq).astype(np.float32)   # [S, 64]
    emb = np.concatenate([freqs, freqs], axis=-1)                   # [S, 128]
    cos = np.cos(emb).astype(np.float32)
    sin = np.sin(emb).astype(np.float32)
    cosT = np.ascontiguousarray(cos.T)                              # [128, S]
    sinT = np.ascontiguousarray(sin.T)
    sinP = sinT.copy()
    sinP[0:64] = -sinP[0:64]

    # causal mask tiles [128, 4, 512] bf16: keep iff 128*d + k_rel <= q
    k_rel = np.arange(P)[:, None, None]
    dd = np.arange(4)[None, :, None]
    qq = np.arange(512)[None, None, :]
    maskT = ((P * dd + k_rel) <= qq).astype(ml_dtypes.bfloat16)

    onesb = np.ones((P, 1), dtype=ml_dtypes.bfloat16)

    in_maps = []
    for c in range(8):
        b, g = c // 2, c % 2
        heads = range(HG * g, HG * g + HG)
        x_b = x[b]                                       # [S, D]
        xT = np.ascontiguousarray(
            x_b.T.reshape(NDM, P, S).transpose(1, 0, 2))  # [128, 16, S]
        # Q then K feature blocks, one per head in group
        blocks = [Wqkv[h * DH:(h + 1) * DH] for h in heads] + \
                 [Wqkv[D + h * DH:D + (h + 1) * DH] for h in heads]
        wqkT = np.stack([
            np.ascontiguousarray(
                blk.T.reshape(NDM, P, P).transpose(1, 0, 2))    # [128, 16, 128]
            for blk in blocks
        ])                                                       # [16, 128, 16, 128]
        Wv = np.concatenate([Wqkv[2 * D + h * DH:2 * D + (h + 1) * DH] for h in heads])
        wvT = np.ascontiguousarray(
            Wv.T.reshape(NDM, P, HG * P).transpose(1, 0, 2))     # [128, 16, 1024]
        Wog = Wo[:, g * HG * DH:(g + 1) * HG * DH]               # [D, 1024]
        woT = np.ascontiguousarray(
            Wog.T.reshape(HG, P, D).transpose(1, 0, 2))          # [128, 8, D]
        in_maps.append({
            'xT': xT, 'wqkT': wqkT, 'wvT': wvT, 'woT': woT,
            'cosT': cosT, 'sinP': sinP, 'maskT': maskT, 'onesb': onesb,
        })
    return in_maps


def kernel(x, Wqkv, Wo):
    from concourse.bass_utils import run_bass_kernel_spmd

    if 'nc' not in _CACHE:
        _CACHE['nc'] = _build_program()
    nc = _CACHE['nc']

    in_maps = _host_inputs(np.asarray(x, dtype=np.float32),
                           np.asarray(Wqkv, dtype=np.float32),
                           np.asarray(Wo, dtype=np.float32))
    res = run_bass_kernel_spmd(nc, in_maps, core_ids=list(range(8)))
    outs = [res.results[c]['out'].reshape(S, D) for c in range(8)]
    full = np.empty((B, S, D), dtype=np.float32)
    for b in range(B):
        full[b] = outs[2 * b] + outs[2 * b + 1]
    return full


# revision 13
# speedup vs baseline: 1.2675x; 1.2675x over previous
"""Distributed causal multi-head attention (RoPE) for 8 TRN2 NeuronCores.

Problem: B=4, S=2048, D=2048, H=16 heads, DH=128.
Sharding: 2D — data-parallel over the 4 batches x tensor-parallel over 2
head-groups of 8 heads (Megatron-style: Wqkv column-sharded per head
group, Wo row-sharded).  Core c handles batch c//2, head group c%2.
Each core returns a partial output projection [S, D]; the host sums the
two group partials per batch (the "all-reduce") and stacks batches.

Per-core pipeline (all matmuls on the PE array):
  stage 1: QKV projection from xT (d-major), RoPE fused into the
           PSUM eviction for Q/K (f32r), V evicted as bf16.
  stage 2: per head: scoresT = K^T-tiles x Q (f32r, N=512), exp via
           ACT (scale=1/sqrt(128)) into bf16 tiles, causal masking by
           multiplying precomputed mask tiles, PV via bf16 matmuls with
           a fused ones-column (denominator), per-q-tile normalization +
           PE transpose into resident zT tiles.
  stage 3: output projection out = sum_h zT_h.T @ WoT_h (f32r, N=512).
"""

import sys

if '/opt/trn_rl_repo' not in sys.path:
    sys.path.insert(0, '/opt/trn_rl_repo')

import math

import ml_dtypes
import numpy as np

B, S, D, H, DH = 4, 2048, 2048, 16, 128
BASE = 10000.0
P = 128
NT = S // P          # 16 token tiles of 128
NC512 = S // 512     # 4 token chunks of 512
NDM = D // P         # 16 d_model chunks
HG = 8               # heads per group
SCALE = 1.0 / math.sqrt(DH)

_CACHE = {}


def _build_program():
    import concourse.bacc as bacc
    import concourse.mybir as mybir
    from concourse.tile import TileContext
    from concourse.masks import make_identity

    F32 = mybir.dt.float32
    F32R = mybir.dt.float32r
    BF16 = mybir.dt.bfloat16
    EXP = mybir.ActivationFunctionType.Exp

    nc = bacc.Bacc('TRN2', target_bir_lowering=False, debug=False, num_devices=8)

    # ---- DRAM I/O ----
    xT = nc.dram_tensor('xT', [P, NDM, S], F32R, kind='ExternalInput').ap()
    wqkT = nc.dram_tensor('wqkT', [2 * HG, P, NDM, P], F32R, kind='ExternalInput').ap()
    wvT = nc.dram_tensor('wvT', [P, NDM, HG * P], F32R, kind='ExternalInput').ap()
    woT = nc.dram_tensor('woT', [P, HG, D], F32R, kind='ExternalInput').ap()
    cosT = nc.dram_tensor('cosT', [P, S], F32, kind='ExternalInput').ap()
    sinP = nc.dram_tensor('sinP', [P, S], F32, kind='ExternalInput').ap()
    maskT = nc.dram_tensor('maskT', [P, P], BF16, kind='ExternalInput').ap()
    onesb = nc.dram_tensor('onesb', [P, 1], BF16, kind='ExternalInput').ap()
    out = nc.dram_tensor('out', [NT, P, D], F32, kind='ExternalOutput').ap()

    # ---- DRAM scratch ----
    qk_scr = nc.dram_tensor('qk_scr', [2 * HG, P, S], F32R).ap()
    v_scr = nc.dram_tensor('v_scr', [NT, P, HG * P], BF16).ap()

    with TileContext(nc) as tc:
        # ================= stage 1: QKV projection =================
        with tc.tile_pool(name='s1x', bufs=1) as xpool:
            xsb = xpool.tile([P, NDM, S], F32R)
            cos_sb = xpool.tile([P, S], F32)
            sin_sb = xpool.tile([P, S], F32)

            # --- Q and K blocks, feature-major (d_head on partitions) ---
            with tc.tile_pool(name='s1w', bufs=2) as wpool, \
                 tc.tile_pool(name='s1e', bufs=3) as epool, \
                 tc.tile_pool(name='s1p', bufs=4, space='PSUM') as ppool:
                for fb in range(2 * HG):
                    wsb = wpool.tile([P, NDM, P], F32R, tag='wqk', name=f'wqk{fb}')
                    nc.sync.dma_start(wsb[:], wqkT[fb])
                    if fb == 0:
                        # x / tables loaded after the first weight block so the
                        # first matmul chain starts as soon as possible
                        for o in range(NDM):
                            nc.sync.dma_start(xsb[:, o, 0:S // 2], xT[:, o, 0:S // 2])
                            nc.sync.dma_start(xsb[:, o, S // 2:S], xT[:, o, S // 2:S])
                        nc.sync.dma_start(cos_sb[:], cosT[:])
                        nc.sync.dma_start(sin_sb[:], sinP[:])
                    for tcn in range(NC512):
                        ts = slice(tcn * 512, tcn * 512 + 512)
                        ps = ppool.tile([P, 512], F32, tag='pqk', name=f'pqk_{fb}_{tcn}')
                        for o in range(NDM):
                            nc.tensor.matmul(ps[:], wsb[:, o, :], xsb[:, o, ts],
                                             start=(o == 0), stop=(o == NDM - 1))
                        # RoPE fused eviction
                        t1 = epool.tile([P, 512], F32, tag='t1', name=f't1_{fb}_{tcn}')
                        t2 = epool.tile([P, 512], F32, tag='t2', name=f't2_{fb}_{tcn}')
                        qt = epool.tile([P, 512], F32R, tag='qt', name=f'qt_{fb}_{tcn}')
                        nc.vector.tensor_mul(t1[:], ps[:], cos_sb[:, ts])
                        # rotate_half via cross-partition reads (sign folded in sinP)
                        nc.vector.tensor_mul(t2[0:64, :], ps[64:128, :], sin_sb[0:64, ts])
                        nc.vector.tensor_mul(t2[64:128, :], ps[0:64, :], sin_sb[64:128, ts])
                        nc.vector.tensor_add(qt[:], t1[:], t2[:])
                        nc.sync.dma_start(qk_scr[fb][:, ts], qt[:])

            # --- V blocks, token-major ---
            with tc.tile_pool(name='s1wv', bufs=2) as wvpool, \
                 tc.tile_pool(name='s1ev', bufs=3) as evpool, \
                 tc.tile_pool(name='s1pv', bufs=4, space='PSUM') as pvpool:
                for vc in range(4):
                    vs = slice(vc * 256, vc * 256 + 256)
                    wv = wvpool.tile([P, NDM, 256], F32R, tag='wv', name=f'wv{vc}')
                    for o in range(NDM):
                        nc.sync.dma_start(wv[:, o, :], wvT[:, o, vs])
                    for tt in range(NT):
                        psv = pvpool.tile([P, 256], F32, tag='pv', name=f'pv_{vc}_{tt}')
                        for o in range(NDM):
                            nc.tensor.matmul(psv[:],
                                             xsb[:, o, tt * P:(tt + 1) * P],
                                             wv[:, o, :],
                                             start=(o == 0), stop=(o == NDM - 1))
                        vsb = evpool.tile([P, 256], BF16, tag='vsb', name=f'vsb_{vc}_{tt}')
                        nc.scalar.copy(vsb[:], psv[:])
                        nc.sync.dma_start(v_scr[tt][:, vs], vsb[:])

        # ================= stage 2: attention =================
        with tc.tile_pool(name='s2c', bufs=1) as cpool, \
             tc.tile_pool(name='s2zt', bufs=1) as ztpool, \
             tc.tile_pool(name='s2qk', bufs=2) as qkpool, \
             tc.tile_pool(name='s2va', bufs=1) as vapool, \
             tc.tile_pool(name='s2st', bufs=2) as stpool, \
             tc.tile_pool(name='s2z', bufs=3) as zpool, \
             tc.tile_pool(name='s2wo', bufs=2) as wopool, \
             tc.tile_pool(name='s2os', bufs=3) as ospool:
            _ps2 = [tc.tile_pool(name='s2p', bufs=3, space='PSUM'),
                    tc.tile_pool(name='s2pz', bufs=2, space='PSUM'),
                    tc.tile_pool(name='s2pt', bufs=2, space='PSUM')]
            sppool, zppool, tppool = [p.__enter__() for p in _ps2]
            msk = cpool.tile([P, P], BF16)
            nc.sync.dma_start(msk[:], maskT[:])
            ident = cpool.tile([P, P], F32)
            make_identity(nc, ident[:])
            ones_sb = cpool.tile([P, 1], BF16)
            nc.sync.dma_start(ones_sb[:], onesb[:])

            zT = [ztpool.tile([P, S], F32R, name=f'zT{h}') for h in range(HG)]

            for h in range(HG):
                qt_h = qkpool.tile([P, S], F32R, tag='qt_h', name=f'qt_h{h}')
                kt_h = qkpool.tile([P, S], F32R, tag='kt_h', name=f'kt_h{h}')
                nc.sync.dma_start(qt_h[:], qk_scr[h])
                nc.sync.dma_start(kt_h[:], qk_scr[HG + h])
                vau = []
                for kt in range(NT):
                    va = vapool.tile([P, P + 1], BF16, tag=f'vau{kt}',
                                     name=f'vau_{h}_{kt}')
                    nc.sync.dma_start(va[:, 0:P], v_scr[kt][:, h * P:(h + 1) * P])
                    nc.vector.tensor_copy(va[:, P:P + 1], ones_sb[:])
                    vau.append(va)

                st = [[None] * NT for _ in range(NC512)]

                def emit_qk(qr, h=h, qt_h=qt_h, kt_h=kt_h, st=st):
                    base = qr * 512
                    for kt in range(4 * qr + 4):
                        d = kt - 4 * qr
                        # causal trim: only columns >= 128*d are ever read;
                        # fp32r needs moving dim >= 256 for full rate.
                        qoff = 0 if d < 0 else min(128 * d, 256)
                        eoff = 0 if d < 0 else 128 * d
                        sps = sppool.tile([P, 512], F32, tag='sps',
                                          name=f'sps_{h}_{qr}_{kt}')
                        nc.tensor.matmul(sps[:, qoff:512],
                                         kt_h[:, kt * P:(kt + 1) * P],
                                         qt_h[:, base + qoff:base + 512],
                                         start=True, stop=True)
                        stt = stpool.tile([P, 512], BF16, tag=f'st{kt}',
                                          name=f'st_{h}_{qr}_{kt}')
                        nc.scalar.activation(stt[:, eoff:512], sps[:, eoff:512],
                                             EXP, scale=SCALE)
                        if d >= 0:
                            # triangular mask on the diagonal 128-block only
                            nc.vector.tensor_mul(stt[:, eoff:eoff + P],
                                                 stt[:, eoff:eoff + P], msk[:])
                        st[qr][kt] = stt

                emit_qk(0)
                for qr in range(NC512):
                    if qr + 1 < NC512:
                        emit_qk(qr + 1)   # QK(qr+1) on PE while ACT exps qr
                    for qs in range(4):
                        qa = 4 * qr + qs
                        zps = zppool.tile([P, P + 1], F32, tag='zps',
                                          name=f'zps_{h}_{qa}')
                        for kt in range(qa + 1):
                            nc.tensor.matmul(zps[:],
                                             st[qr][kt][:, qs * P:(qs + 1) * P],
                                             vau[kt][:],
                                             start=(kt == 0), stop=(kt == qa))
                        rcp = zpool.tile([P, 1], F32, tag='rcp',
                                         name=f'rcp_{h}_{qa}')
                        nc.vector.reciprocal(rcp[:], zps[:, P:P + 1])
                        zsb = zpool.tile([P, P], F32, tag='zsb',
                                         name=f'zsb_{h}_{qa}')
                        nc.vector.tensor_scalar_mul(zsb[:], zps[:, 0:P], rcp[:])
                        ztp = tppool.tile([P, P], F32, tag='ztp',
                                          name=f'ztp_{h}_{qa}')
                        nc.tensor.transpose(ztp[:], zsb[:], ident[:])
                        nc.vector.tensor_copy(zT[h][:, qa * P:(qa + 1) * P],
                                              ztp[:])

            for p in reversed(_ps2):
                p.__exit__(None, None, None)

            # ================= stage 3: output projection =================
            with tc.tile_pool(name='s3p', bufs=3, space='PSUM') as oppool:
                for ec in range(NC512):
                    es = slice(ec * 512, ec * 512 + 512)
                    wo = wopool.tile([P, HG, 512], F32R, tag='wo', name=f'wo{ec}')
                    for h in range(HG):
                        nc.sync.dma_start(wo[:, h, :], woT[:, h, es])
                    for tt in range(NT):
                        pso = oppool.tile([P, 512], F32, tag='pso',
                                          name=f'pso_{tt}_{ec}')
                        for h in range(HG):
                            nc.tensor.matmul(pso[:], zT[h][:, tt * P:(tt + 1) * P],
                                             wo[:, h, :],
                                             start=(h == 0), stop=(h == HG - 1))
                        osb = ospool.tile([P, 512], F32, tag='osb',
                                          name=f'osb_{tt}_{ec}')
                        if tt % 2 == 0:
                            nc.scalar.copy(osb[:], pso[:])
                        else:
                            nc.vector.tensor_copy(osb[:], pso[:])
                        nc.sync.dma_start(out[tt][:, es], osb[:])

    nc.compile()
    return nc


def _host_inputs(x, Wqkv, Wo):
    """Build the 8 per-core input maps."""
    # RoPE tables (match reference: float32 math)
    inv_freq = (1.0 / (BASE ** (np.arange(0, DH, 2, dtype=np.float32) / DH))).astype(np.float32)
    t = np.arange(S, dtype=np.float32)
    freqs = np.einsum('i,j->ij', t, inv_freq).astype(np.float32)   # [S, 64]
    emb = np.concatenate([freqs, freqs], axis=-1)                   # [S, 128]
    cos = np.cos(emb).astype(np.float32)
    sin = np.sin(emb).astype(np.float32)
    cosT = np.ascontiguousarray(cos.T)                              # [128, S]
    sinT = np.ascontiguousarray(sin.T)
    sinP = sinT.copy()
    sinP[0:64] = -sinP[0:64]

    # triangular causal mask [128, 128] bf16: keep iff k_rel <= q_rel
    maskT = (np.arange(P)[:, None] <= np.arange(P)[None, :]).astype(ml_dtypes.bfloat16)

    onesb = np.ones((P, 1), dtype=ml_dtypes.bfloat16)

    in_maps = []
    for c in range(8):
        b, g = c // 2, c % 2
        heads = range(HG * g, HG * g + HG)
        x_b = x[b]                                       # [S, D]
        xT = np.ascontiguousarray(
            x_b.T.reshape(NDM, P, S).transpose(1, 0, 2))  # [128, 16, S]
        # Q then K feature blocks, one per head in group
        blocks = [Wqkv[h * DH:(h + 1) * DH] for h in heads] + \
                 [Wqkv[D + h * DH:D + (h + 1) * DH] for h in heads]
        wqkT = np.stack([
            np.ascontiguousarray(
                blk.T.reshape(NDM, P, P).transpose(1, 0, 2))    # [128, 16, 128]
            for blk in blocks
        ])                                                       # [16, 128, 16, 128]
        Wv = np.concatenate([Wqkv[2 * D + h * DH:2 * D + (h + 1) * DH] for h in heads])
        wvT = np.ascontiguousarray(
            Wv.T.reshape(NDM, P, HG * P).transpose(1, 0, 2))     # [128, 16, 1024]
        Wog = Wo[:, g * HG * DH:(g + 1) * HG * DH]               # [D, 1024]
        woT = np.ascontiguousarray(
            Wog.T.reshape(HG, P, D).transpose(1, 0, 2))          # [128, 8, D]
        in_maps.append({
            'xT': xT, 'wqkT': wqkT, 'wvT': wvT, 'woT': woT,
            'cosT': cosT, 'sinP': sinP, 'maskT': maskT, 'onesb': onesb,
        })
    return in_maps


def kernel(x, Wqkv, Wo):
    from concourse.bass_utils import run_bass_kernel_spmd

    if 'nc' not in _CACHE:
        _CACHE['nc'] = _build_program()
    nc = _CACHE['nc']

    in_maps = _host_inputs(np.asarray(x, dtype=np.float32),
                           np.asarray(Wqkv, dtype=np.float32),
                           np.asarray(Wo, dtype=np.float32))
    res = run_bass_kernel_spmd(nc, in_maps, core_ids=list(range(8)))
    outs = [res.results[c]['out'].reshape(S, D) for c in range(8)]
    full = np.empty((B, S, D), dtype=np.float32)
    for b in range(B):
        full[b] = outs[2 * b] + outs[2 * b + 1]
    return full


# revision 14
# speedup vs baseline: 1.2917x; 1.0191x over previous
"""Distributed causal multi-head attention (RoPE) for 8 TRN2 NeuronCores.

Problem: B=4, S=2048, D=2048, H=16 heads, DH=128.
Sharding: 2D — data-parallel over the 4 batches x tensor-parallel over 2
head-groups of 8 heads (Megatron-style: Wqkv column-sharded per head
group, Wo row-sharded).  Core c handles batch c//2, head group c%2.
Each core returns a partial output projection [S, D]; the host sums the
two group partials per batch (the "all-reduce") and stacks batches.

Per-core pipeline (all matmuls on the PE array):
  stage 1: QKV projection from xT (d-major), RoPE fused into the
           PSUM eviction for Q/K (f32r), V evicted as bf16.
  stage 2: per head: scoresT = K^T-tiles x Q (f32r, N=512), exp via
           ACT (scale=1/sqrt(128)) into bf16 tiles, causal masking by
           multiplying precomputed mask tiles, PV via bf16 matmuls with
           a fused ones-column (denominator), per-q-tile normalization +
           PE transpose into resident zT tiles.
  stage 3: output projection out = sum_h zT_h.T @ WoT_h (f32r, N=512).
"""

import sys

if '/opt/trn_rl_repo' not in sys.path:
    sys.path.insert(0, '/opt/trn_rl_repo')

import math

import ml_dtypes
import numpy as np

B, S, D, H, DH = 4, 2048, 2048, 16, 128
BASE = 10000.0
P = 128
NT = S // P          # 16 token tiles of 128
NC512 = S // 512     # 4 token chunks of 512
NDM = D // P         # 16 d_model chunks
HG = 8               # heads per group
SCALE = 1.0 / math.sqrt(DH)

_CACHE = {}


def _build_program():
    import concourse.bacc as bacc
    import concourse.mybir as mybir
    from concourse.tile import TileContext
    from concourse.masks import make_identity

    F32 = mybir.dt.float32
    F32R = mybir.dt.float32r
    BF16 = mybir.dt.bfloat16
    EXP = mybir.ActivationFunctionType.Exp

    nc = bacc.Bacc('TRN2', target_bir_lowering=False, debug=False, num_devices=8)

    # ---- DRAM I/O ----
    xT = nc.dram_tensor('xT', [P, NDM, S], F32R, kind='ExternalInput').ap()
    wqkT = nc.dram_tensor('wqkT', [2 * HG, P, NDM, P], F32R, kind='ExternalInput').ap()
    wvT = nc.dram_tensor('wvT', [P, NDM, HG * P], F32R, kind='ExternalInput').ap()
    woT = nc.dram_tensor('woT', [P, HG, D], F32R, kind='ExternalInput').ap()
    cosT = nc.dram_tensor('cosT', [P, S], F32, kind='ExternalInput').ap()
    sinP = nc.dram_tensor('sinP', [P, S], F32, kind='ExternalInput').ap()
    maskT = nc.dram_tensor('maskT', [P, P], BF16, kind='ExternalInput').ap()
    onesb = nc.dram_tensor('onesb', [P, 1], BF16, kind='ExternalInput').ap()
    out = nc.dram_tensor('out', [NT, P, D], F32, kind='ExternalOutput').ap()

    # ---- DRAM scratch ----
    qk_scr = nc.dram_tensor('qk_scr', [2 * HG, P, S], F32R).ap()
    v_scr = nc.dram_tensor('v_scr', [NT, P, HG * P], BF16).ap()

    with TileContext(nc) as tc:
        # ================= stage 1: QKV projection =================
        with tc.tile_pool(name='s1x', bufs=1) as xpool:
            xsb = xpool.tile([P, NDM, S], F32R)
            cos_sb = xpool.tile([P, S], F32)
            sin_sb = xpool.tile([P, S], F32)

            # --- Q and K blocks, feature-major (d_head on partitions) ---
            with tc.tile_pool(name='s1w', bufs=2) as wpool, \
                 tc.tile_pool(name='s1e', bufs=3) as epool, \
                 tc.tile_pool(name='s1p', bufs=4, space='PSUM') as ppool:
                for fb in range(2 * HG):
                    wsb = wpool.tile([P, NDM, P], F32R, tag='wqk', name=f'wqk{fb}')
                    nc.sync.dma_start(wsb[:], wqkT[fb])
                    if fb == 0:
                        # x / tables loaded after the first weight block so the
                        # first matmul chain starts as soon as possible
                        for o in range(NDM):
                            nc.sync.dma_start(xsb[:, o, 0:S // 2], xT[:, o, 0:S // 2])
                            nc.sync.dma_start(xsb[:, o, S // 2:S], xT[:, o, S // 2:S])
                        nc.sync.dma_start(cos_sb[:], cosT[:])
                        nc.sync.dma_start(sin_sb[:], sinP[:])
                    for tcn in range(NC512):
                        ts = slice(tcn * 512, tcn * 512 + 512)
                        ps = ppool.tile([P, 512], F32, tag='pqk', name=f'pqk_{fb}_{tcn}')
                        for o in range(NDM):
                            nc.tensor.matmul(ps[:], wsb[:, o, :], xsb[:, o, ts],
                                             start=(o == 0), stop=(o == NDM - 1))
                        # RoPE fused eviction
                        t1 = epool.tile([P, 512], F32, tag='t1', name=f't1_{fb}_{tcn}')
                        t2 = epool.tile([P, 512], F32, tag='t2', name=f't2_{fb}_{tcn}')
                        qt = epool.tile([P, 512], F32R, tag='qt', name=f'qt_{fb}_{tcn}')
                        nc.vector.tensor_mul(t1[:], ps[:], cos_sb[:, ts])
                        # rotate_half via cross-partition reads (sign folded in sinP)
                        nc.vector.tensor_mul(t2[0:64, :], ps[64:128, :], sin_sb[0:64, ts])
                        nc.vector.tensor_mul(t2[64:128, :], ps[0:64, :], sin_sb[64:128, ts])
                        nc.vector.tensor_add(qt[:], t1[:], t2[:])
                        nc.sync.dma_start(qk_scr[fb][:, ts], qt[:])

            # --- V blocks, token-major ---
            with tc.tile_pool(name='s1wv', bufs=2) as wvpool, \
                 tc.tile_pool(name='s1ev', bufs=3) as evpool, \
                 tc.tile_pool(name='s1pv', bufs=4, space='PSUM') as pvpool:
                for vc in range(4):
                    vs = slice(vc * 256, vc * 256 + 256)
                    wv = wvpool.tile([P, NDM, 256], F32R, tag='wv', name=f'wv{vc}')
                    for o in range(NDM):
                        nc.sync.dma_start(wv[:, o, :], wvT[:, o, vs])
                    for tt in range(NT):
                        psv = pvpool.tile([P, 256], F32, tag='pv', name=f'pv_{vc}_{tt}')
                        for o in range(NDM):
                            nc.tensor.matmul(psv[:],
                                             xsb[:, o, tt * P:(tt + 1) * P],
                                             wv[:, o, :],
                                             start=(o == 0), stop=(o == NDM - 1))
                        vsb = evpool.tile([P, 256], BF16, tag='vsb', name=f'vsb_{vc}_{tt}')
                        nc.scalar.copy(vsb[:], psv[:])
                        nc.sync.dma_start(v_scr[tt][:, vs], vsb[:])

        # ================= stage 2: attention =================
        with tc.tile_pool(name='s2c', bufs=1) as cpool, \
             tc.tile_pool(name='s2zt', bufs=1) as ztpool, \
             tc.tile_pool(name='s2qk', bufs=2) as qkpool, \
             tc.tile_pool(name='s2va', bufs=2) as vapool, \
             tc.tile_pool(name='s2st', bufs=2) as stpool, \
             tc.tile_pool(name='s2z', bufs=3) as zpool, \
             tc.tile_pool(name='s2wo', bufs=2) as wopool, \
             tc.tile_pool(name='s2os', bufs=3) as ospool:
            _ps2 = [tc.tile_pool(name='s2p', bufs=4, space='PSUM'),
                    tc.tile_pool(name='s2pz', bufs=2, space='PSUM'),
                    tc.tile_pool(name='s2pt', bufs=2, space='PSUM')]
            sppool, zppool, tppool = [p.__enter__() for p in _ps2]
            msk = cpool.tile([P, P], BF16)
            nc.sync.dma_start(msk[:], maskT[:])
            ident = cpool.tile([P, P], F32)
            make_identity(nc, ident[:])
            ones_sb = cpool.tile([P, 1], BF16)
            nc.sync.dma_start(ones_sb[:], onesb[:])

            zT = [ztpool.tile([P, S], F32R, name=f'zT{h}') for h in range(HG)]

            for h in range(HG):
                qt_h = qkpool.tile([P, S], F32R, tag='qt_h', name=f'qt_h{h}')
                kt_h = qkpool.tile([P, S], F32R, tag='kt_h', name=f'kt_h{h}')
                nc.sync.dma_start(qt_h[:], qk_scr[h])
                nc.sync.dma_start(kt_h[:], qk_scr[HG + h])
                vau = []
                for kt in range(NT):
                    va = vapool.tile([P, P + 1], BF16, tag=f'vau{kt}',
                                     name=f'vau_{h}_{kt}')
                    nc.sync.dma_start(va[:, 0:P], v_scr[kt][:, h * P:(h + 1) * P])
                    nc.vector.tensor_copy(va[:, P:P + 1], ones_sb[:])
                    vau.append(va)

                st = [[None] * NT for _ in range(NC512)]

                def emit_qk(qr, h=h, qt_h=qt_h, kt_h=kt_h, st=st):
                    base = qr * 512
                    for kt in range(4 * qr + 4):
                        d = kt - 4 * qr
                        # causal trim: only columns >= 128*d are ever read;
                        # fp32r needs moving dim >= 256 for full rate.
                        qoff = 0 if d < 0 else min(128 * d, 256)
                        eoff = 0 if d < 0 else 128 * d
                        sps = sppool.tile([P, 512], F32, tag='sps',
                                          name=f'sps_{h}_{qr}_{kt}')
                        nc.tensor.matmul(sps[:, qoff:512],
                                         kt_h[:, kt * P:(kt + 1) * P],
                                         qt_h[:, base + qoff:base + 512],
                                         start=True, stop=True)
                        stt = stpool.tile([P, 512], BF16, tag=f'st{kt}',
                                          name=f'st_{h}_{qr}_{kt}')
                        nc.scalar.activation(stt[:, eoff:512], sps[:, eoff:512],
                                             EXP, scale=SCALE)
                        if d >= 0:
                            # triangular mask on the diagonal 128-block only
                            nc.vector.tensor_mul(stt[:, eoff:eoff + P],
                                                 stt[:, eoff:eoff + P], msk[:])
                        st[qr][kt] = stt

                emit_qk(0)
                for qr in range(NC512):
                    if qr + 1 < NC512:
                        emit_qk(qr + 1)   # QK(qr+1) on PE while ACT exps qr
                    for qs in range(4):
                        qa = 4 * qr + qs
                        zps = zppool.tile([P, P + 1], F32, tag='zps',
                                          name=f'zps_{h}_{qa}')
                        for kt in range(qa + 1):
                            nc.tensor.matmul(zps[:],
                                             st[qr][kt][:, qs * P:(qs + 1) * P],
                                             vau[kt][:],
                                             start=(kt == 0), stop=(kt == qa))
                        rcp = zpool.tile([P, 1], F32, tag='rcp',
                                         name=f'rcp_{h}_{qa}')
                        nc.vector.reciprocal(rcp[:], zps[:, P:P + 1])
                        zsb = zpool.tile([P, P], F32, tag='zsb',
                                         name=f'zsb_{h}_{qa}')
                        nc.vector.tensor_scalar_mul(zsb[:], zps[:, 0:P], rcp[:])
                        ztp = tppool.tile([P, P], F32, tag='ztp',
                                          name=f'ztp_{h}_{qa}')
                        nc.tensor.transpose(ztp[:], zsb[:], ident[:])
                        nc.vector.tensor_copy(zT[h][:, qa * P:(qa + 1) * P],
                                              ztp[:])

            for p in reversed(_ps2):
                p.__exit__(None, None, None)

            # ================= stage 3: output projection =================
            with tc.tile_pool(name='s3p', bufs=3, space='PSUM') as oppool:
                for ec in range(NC512):
                    es = slice(ec * 512, ec * 512 + 512)
                    wo = wopool.tile([P, HG, 512], F32R, tag='wo', name=f'wo{ec}')
                    for h in range(HG):
                        nc.sync.dma_start(wo[:, h, :], woT[:, h, es])
                    for tt in range(NT):
                        pso = oppool.tile([P, 512], F32, tag='pso',
                                          name=f'pso_{tt}_{ec}')
                        for h in range(HG):
                            nc.tensor.matmul(pso[:], zT[h][:, tt * P:(tt + 1) * P],
                                             wo[:, h, :],
                                             start=(h == 0), stop=(h == HG - 1))
                        osb = ospool.tile([P, 512], F32, tag='osb',
                                          name=f'osb_{tt}_{ec}')
                        if tt % 2 == 0:
                            nc.scalar.copy(osb[:], pso[:])
                        else:
                            nc.vector.tensor_copy(osb[:], pso[:])
                        nc.sync.dma_start(out[tt][:, es], osb[:])

    nc.compile()
    return nc


def _host_inputs(x, Wqkv, Wo):
    """Build the 8 per-core input maps."""
    # RoPE tables (match reference: float32 math)
    inv_freq = (1.0 / (BASE ** (np.arange(0, DH, 2, dtype=np.float32) / DH))).astype(np.float32)
    t = np.arange(S, dtype=np.float32)
    freqs = np.einsum('i,j->ij', t, inv_freq).astype(np.float32)   # [S, 64]
    emb = np.concatenate([freqs, freqs], axis=-1)                   # [S, 128]
    cos = np.cos(emb).astype(np.float32)
    sin = np.sin(emb).astype(np.float32)
    cosT = np.ascontiguousarray(cos.T)                              # [128, S]
    sinT = np.ascontiguousarray(sin.T)
    sinP = sinT.copy()
    sinP[0:64] = -sinP[0:64]

    # triangular causal mask [128, 128] bf16: keep iff k_rel <= q_rel
    maskT = (np.arange(P)[:, None] <= np.arange(P)[None, :]).astype(ml_dtypes.bfloat16)

    onesb = np.ones((P, 1), dtype=ml_dtypes.bfloat16)

    in_maps = []
    for c in range(8):
        b, g = c // 2, c % 2
        heads = range(HG * g, HG * g + HG)
        x_b = x[b]                                       # [S, D]
        xT = np.ascontiguousarray(
            x_b.T.reshape(NDM, P, S).transpose(1, 0, 2))  # [128, 16, S]
        # Q then K feature blocks, one per head in group
        blocks = [Wqkv[h * DH:(h + 1) * DH] for h in heads] + \
                 [Wqkv[D + h * DH:D + (h + 1) * DH] for h in heads]
        wqkT = np.stack([
            np.ascontiguousarray(
                blk.T.reshape(NDM, P, P).transpose(1, 0, 2))    # [128, 16, 128]
            for blk in blocks
        ])                                                       # [16, 128, 16, 128]
        Wv = np.concatenate([Wqkv[2 * D + h * DH:2 * D + (h + 1) * DH] for h in heads])
        wvT = np.ascontiguousarray(
            Wv.T.reshape(NDM, P, HG * P).transpose(1, 0, 2))     # [128, 16, 1024]
        Wog = Wo[:, g * HG * DH:(g + 1) * HG * DH]               # [D, 1024]
        woT = np.ascontiguousarray(
            Wog.T.reshape(HG, P, D).transpose(1, 0, 2))          # [128, 8, D]
        in_maps.append({
            'xT': xT, 'wqkT': wqkT, 'wvT': wvT, 'woT': woT,
            'cosT': cosT, 'sinP': sinP, 'maskT': maskT, 'onesb': onesb,
        })
    return in_maps


def kernel(x, Wqkv, Wo):
    from concourse.bass_utils import run_bass_kernel_spmd

    if 'nc' not in _CACHE:
        _CACHE['nc'] = _build_program()
    nc = _CACHE['nc']

    in_maps = _host_inputs(np.asarray(x, dtype=np.float32),
                           np.asarray(Wqkv, dtype=np.float32),
                           np.asarray(Wo, dtype=np.float32))
    res = run_bass_kernel_spmd(nc, in_maps, core_ids=list(range(8)))
    outs = [res.results[c]['out'].reshape(S, D) for c in range(8)]
    full = np.empty((B, S, D), dtype=np.float32)
    for b in range(B):
        full[b] = outs[2 * b] + outs[2 * b + 1]
    return full


# revision 15
# speedup vs baseline: 1.3037x; 1.0093x over previous
"""Distributed causal multi-head attention (RoPE) for 8 TRN2 NeuronCores.

Problem: B=4, S=2048, D=2048, H=16 heads, DH=128.
Sharding: 2D — data-parallel over the 4 batches x tensor-parallel over 2
head-groups of 8 heads (Megatron-style: Wqkv column-sharded per head
group, Wo row-sharded).  Core c handles batch c//2, head group c%2.
Each core returns a partial output projection [S, D]; the host sums the
two group partials per batch (the "all-reduce") and stacks batches.

Per-core pipeline (all matmuls on the PE array):
  stage 1: QKV projection from xT (d-major), RoPE fused into the
           PSUM eviction for Q/K (f32r), V evicted as bf16.
  stage 2: per head: scoresT = K^T-tiles x Q (f32r, N=512), exp via
           ACT (scale=1/sqrt(128)) into bf16 tiles, causal masking by
           multiplying precomputed mask tiles, PV via bf16 matmuls with
           a fused ones-column (denominator), per-q-tile normalization +
           PE transpose into resident zT tiles.
  stage 3: output projection out = sum_h zT_h.T @ WoT_h (f32r, N=512).
"""

import sys

if '/opt/trn_rl_repo' not in sys.path:
    sys.path.insert(0, '/opt/trn_rl_repo')

import math

import ml_dtypes
import numpy as np

B, S, D, H, DH = 4, 2048, 2048, 16, 128
BASE = 10000.0
P = 128
NT = S // P          # 16 token tiles of 128
NC512 = S // 512     # 4 token chunks of 512
NDM = D // P         # 16 d_model chunks
HG = 8               # heads per group
SCALE = 1.0 / math.sqrt(DH)

_CACHE = {}


def _build_program():
    import concourse.bacc as bacc
    import concourse.mybir as mybir
    from concourse.tile import TileContext
    from concourse.masks import make_identity

    F32 = mybir.dt.float32
    F32R = mybir.dt.float32r
    BF16 = mybir.dt.bfloat16
    EXP = mybir.ActivationFunctionType.Exp

    nc = bacc.Bacc('TRN2', target_bir_lowering=False, debug=False, num_devices=8)

    # ---- DRAM I/O ----
    xT = nc.dram_tensor('xT', [P, NDM, S], F32R, kind='ExternalInput').ap()
    wqkT = nc.dram_tensor('wqkT', [2 * HG, P, NDM, P], F32R, kind='ExternalInput').ap()
    wvT = nc.dram_tensor('wvT', [P, NDM, HG * P], F32R, kind='ExternalInput').ap()
    woT = nc.dram_tensor('woT', [P, HG, D], F32R, kind='ExternalInput').ap()
    cosT = nc.dram_tensor('cosT', [P, S], F32, kind='ExternalInput').ap()
    sinP = nc.dram_tensor('sinP', [P, S], F32, kind='ExternalInput').ap()
    maskT = nc.dram_tensor('maskT', [P, P], BF16, kind='ExternalInput').ap()
    onesb = nc.dram_tensor('onesb', [P, 1], BF16, kind='ExternalInput').ap()
    out = nc.dram_tensor('out', [NT, P, D], F32, kind='ExternalOutput').ap()

    # ---- DRAM scratch ----
    qk_scr = nc.dram_tensor('qk_scr', [2 * HG, P, S], F32R).ap()
    v_scr = nc.dram_tensor('v_scr', [NT, P, HG * P], BF16).ap()

    with TileContext(nc) as tc:
        # ================= stage 1: QKV projection =================
        with tc.tile_pool(name='s1x', bufs=1) as xpool:
            xsb = xpool.tile([P, NDM, S], F32R)
            cos_sb = xpool.tile([P, S], F32)
            sin_sb = xpool.tile([P, S], F32)

            # --- shared stage-1 pools: wqk and wv share one sized tag ---
            with tc.tile_pool(name='s1w', bufs=2) as wpool, \
                 tc.tile_pool(name='s1e', bufs=2) as epool, \
                 tc.tile_pool(name='s1ev', bufs=3) as evpool, \
                 tc.tile_pool(name='s1p', bufs=4, space='PSUM') as ppool:
                for fb in range(2 * HG):
                    wsb = wpool.tile([P, NDM, 256], F32R, tag='w', name=f'wqk{fb}')
                    nc.sync.dma_start(wsb[:, :, 0:P], wqkT[fb])
                    if fb == 0:
                        # x / tables loaded after the first weight block so the
                        # first matmul chain starts as soon as possible
                        for o in range(NDM):
                            nc.sync.dma_start(xsb[:, o, 0:S // 2], xT[:, o, 0:S // 2])
                            nc.sync.dma_start(xsb[:, o, S // 2:S], xT[:, o, S // 2:S])
                        nc.sync.dma_start(cos_sb[:], cosT[:])
                        nc.sync.dma_start(sin_sb[:], sinP[:])
                    for tcn in range(NC512):
                        ts = slice(tcn * 512, tcn * 512 + 512)
                        ps = ppool.tile([P, 512], F32, tag='pqk', name=f'pqk_{fb}_{tcn}')
                        for o in range(NDM):
                            nc.tensor.matmul(ps[:], wsb[:, o, 0:P], xsb[:, o, ts],
                                             start=(o == 0), stop=(o == NDM - 1))
                        # RoPE fused eviction
                        t1 = epool.tile([P, 512], F32, tag='t1', name=f't1_{fb}_{tcn}')
                        t2 = epool.tile([P, 512], F32, tag='t2', name=f't2_{fb}_{tcn}')
                        qt = epool.tile([P, 512], F32R, tag='qt', name=f'qt_{fb}_{tcn}')
                        nc.vector.tensor_mul(t1[:], ps[:], cos_sb[:, ts])
                        # rotate_half via cross-partition reads (sign folded in sinP)
                        nc.vector.tensor_mul(t2[0:64, :], ps[64:128, :], sin_sb[0:64, ts])
                        nc.vector.tensor_mul(t2[64:128, :], ps[0:64, :], sin_sb[64:128, ts])
                        nc.vector.tensor_add(qt[:], t1[:], t2[:])
                        nc.sync.dma_start(qk_scr[fb][:, ts], qt[:])

                # --- V blocks, token-major (same pools, no phase boundary) ---
                for vc in range(4):
                    vs = slice(vc * 256, vc * 256 + 256)
                    wv = wpool.tile([P, NDM, 256], F32R, tag='w', name=f'wv{vc}')
                    for o in range(NDM):
                        nc.sync.dma_start(wv[:, o, :], wvT[:, o, vs])
                    for tt in range(NT):
                        psv = ppool.tile([P, 256], F32, tag='pv', name=f'pv_{vc}_{tt}')
                        for o in range(NDM):
                            nc.tensor.matmul(psv[:],
                                             xsb[:, o, tt * P:(tt + 1) * P],
                                             wv[:, o, :],
                                             start=(o == 0), stop=(o == NDM - 1))
                        vsb = evpool.tile([P, 256], BF16, tag='vsb', name=f'vsb_{vc}_{tt}')
                        nc.scalar.copy(vsb[:], psv[:])
                        nc.sync.dma_start(v_scr[tt][:, vs], vsb[:])

        # ================= stage 2: attention =================
        with tc.tile_pool(name='s2c', bufs=1) as cpool, \
             tc.tile_pool(name='s2zt', bufs=1) as ztpool, \
             tc.tile_pool(name='s2qk', bufs=2) as qkpool, \
             tc.tile_pool(name='s2va', bufs=2) as vapool, \
             tc.tile_pool(name='s2st', bufs=2) as stpool, \
             tc.tile_pool(name='s2z', bufs=3) as zpool, \
             tc.tile_pool(name='s2wo', bufs=2) as wopool, \
             tc.tile_pool(name='s2os', bufs=3) as ospool:
            _ps2 = [tc.tile_pool(name='s2p', bufs=4, space='PSUM'),
                    tc.tile_pool(name='s2pz', bufs=2, space='PSUM'),
                    tc.tile_pool(name='s2pt', bufs=2, space='PSUM')]
            sppool, zppool, tppool = [p.__enter__() for p in _ps2]
            msk = cpool.tile([P, P], BF16)
            nc.sync.dma_start(msk[:], maskT[:])
            ident = cpool.tile([P, P], F32)
            make_identity(nc, ident[:])
            ones_sb = cpool.tile([P, 1], BF16)
            nc.sync.dma_start(ones_sb[:], onesb[:])

            zT = [ztpool.tile([P, S], F32R, name=f'zT{h}') for h in range(HG)]

            for h in range(HG):
                qt_h = qkpool.tile([P, S], F32R, tag='qt_h', name=f'qt_h{h}')
                kt_h = qkpool.tile([P, S], F32R, tag='kt_h', name=f'kt_h{h}')
                nc.sync.dma_start(qt_h[:], qk_scr[h])
                nc.sync.dma_start(kt_h[:], qk_scr[HG + h])
                vau = []
                for kt in range(NT):
                    va = vapool.tile([P, P + 1], BF16, tag=f'vau{kt}',
                                     name=f'vau_{h}_{kt}')
                    nc.sync.dma_start(va[:, 0:P], v_scr[kt][:, h * P:(h + 1) * P])
                    nc.vector.tensor_copy(va[:, P:P + 1], ones_sb[:])
                    vau.append(va)

                st = [[None] * NT for _ in range(NC512)]

                def emit_qk(qr, h=h, qt_h=qt_h, kt_h=kt_h, st=st):
                    base = qr * 512
                    for kt in range(4 * qr + 4):
                        d = kt - 4 * qr
                        # causal trim: only columns >= 128*d are ever read;
                        # fp32r needs moving dim >= 256 for full rate.
                        qoff = 0 if d < 0 else min(128 * d, 256)
                        eoff = 0 if d < 0 else 128 * d
                        sps = sppool.tile([P, 512], F32, tag='sps',
                                          name=f'sps_{h}_{qr}_{kt}')
                        nc.tensor.matmul(sps[:, qoff:512],
                                         kt_h[:, kt * P:(kt + 1) * P],
                                         qt_h[:, base + qoff:base + 512],
                                         start=True, stop=True)
                        stt = stpool.tile([P, 512], BF16, tag=f'st{kt}',
                                          name=f'st_{h}_{qr}_{kt}')
                        nc.scalar.activation(stt[:, eoff:512], sps[:, eoff:512],
                                             EXP, scale=SCALE)
                        if d >= 0:
                            # triangular mask on the diagonal 128-block only
                            nc.vector.tensor_mul(stt[:, eoff:eoff + P],
                                                 stt[:, eoff:eoff + P], msk[:])
                        st[qr][kt] = stt

                emit_qk(0)
                for qr in range(NC512):
                    if qr + 1 < NC512:
                        emit_qk(qr + 1)   # QK(qr+1) on PE while ACT exps qr
                    for qs in range(4):
                        qa = 4 * qr + qs
                        zps = zppool.tile([P, P + 1], F32, tag='zps',
                                          name=f'zps_{h}_{qa}')
                        for kt in range(qa + 1):
                            nc.tensor.matmul(zps[:],
                                             st[qr][kt][:, qs * P:(qs + 1) * P],
                                             vau[kt][:],
                                             start=(kt == 0), stop=(kt == qa))
                        rcp = zpool.tile([P, 1], F32, tag='rcp',
                                         name=f'rcp_{h}_{qa}')
                        nc.vector.reciprocal(rcp[:], zps[:, P:P + 1])
                        zsb = zpool.tile([P, P], F32, tag='zsb',
                                         name=f'zsb_{h}_{qa}')
                        nc.vector.tensor_scalar_mul(zsb[:], zps[:, 0:P], rcp[:])
                        ztp = tppool.tile([P, P], F32, tag='ztp',
                                          name=f'ztp_{h}_{qa}')
                        nc.tensor.transpose(ztp[:], zsb[:], ident[:])
                        nc.vector.tensor_copy(zT[h][:, qa * P:(qa + 1) * P],
                                              ztp[:])

            for p in reversed(_ps2):
                p.__exit__(None, None, None)

            # ================= stage 3: output projection =================
            with tc.tile_pool(name='s3p', bufs=3, space='PSUM') as oppool:
                for ec in range(NC512):
                    es = slice(ec * 512, ec * 512 + 512)
                    wo = wopool.tile([P, HG, 512], F32R, tag='wo', name=f'wo{ec}')
                    for h in range(HG):
                        nc.sync.dma_start(wo[:, h, :], woT[:, h, es])
                    for tt in range(NT):
                        pso = oppool.tile([P, 512], F32, tag='pso',
                                          name=f'pso_{tt}_{ec}')
                        for h in range(HG):
                            nc.tensor.matmul(pso[:], zT[h][:, tt * P:(tt + 1) * P],
                                             wo[:, h, :],
                                             start=(h == 0), stop=(h == HG - 1))
                        osb = ospool.tile([P, 512], F32, tag='osb',
                                          name=f'osb_{tt}_{ec}')
                        if tt % 2 == 0:
                            nc.scalar.copy(osb[:], pso[:])
                        else:
                            nc.vector.tensor_copy(osb[:], pso[:])
                        nc.sync.dma_start(out[tt][:, es], osb[:])

    nc.compile()
    return nc


def _host_inputs(x, Wqkv, Wo):
    """Build the 8 per-core input maps."""
    # RoPE tables (match reference: float32 math)
    inv_freq = (1.0 / (BASE ** (np.arange(0, DH, 2, dtype=np.float32) / DH))).astype(np.float32)
    t = np.arange(S, dtype=np.float32)
    freqs = np.einsum('i,j->ij', t, inv_freq).astype(np.float32)   # [S, 64]
    emb = np.concatenate([freqs, freqs], axis=-1)                   # [S, 128]
    cos = np.cos(emb).astype(np.float32)
    sin = np.sin(emb).astype(np.float32)
    cosT = np.ascontiguousarray(cos.T)                              # [128, S]
    sinT = np.ascontiguousarray(sin.T)
    sinP = sinT.copy()
    sinP[0:64] = -sinP[0:64]

    # triangular causal mask [128, 128] bf16: keep iff k_rel <= q_rel
    maskT = (np.arange(P)[:, None] <= np.arange(P)[None, :]).astype(ml_dtypes.bfloat16)

    onesb = np.ones((P, 1), dtype=ml_dtypes.bfloat16)

    in_maps = []
    for c in range(8):
        b, g = c // 2, c % 2
        heads = range(HG * g, HG * g + HG)
        x_b = x[b]                                       # [S, D]
        xT = np.ascontiguousarray(
            x_b.T.reshape(NDM, P, S).transpose(1, 0, 2))  # [128, 16, S]
        # Q then K feature blocks, one per head in group
        blocks = [Wqkv[h * DH:(h + 1) * DH] for h in heads] + \
                 [Wqkv[D + h * DH:D + (h + 1) * DH] for h in heads]
        wqkT = np.stack([
            np.ascontiguousarray(
                blk.T.reshape(NDM, P, P).transpose(1, 0, 2))    # [128, 16, 128]
            for blk in blocks
        ])                                                       # [16, 128, 16, 128]
        Wv = np.concatenate([Wqkv[2 * D + h * DH:2 * D + (h + 1) * DH] for h in heads])
        wvT = np.ascontiguousarray(
            Wv.T.reshape(NDM, P, HG * P).transpose(1, 0, 2))     # [128, 16, 1024]
        Wog = Wo[:, g * HG * DH:(g + 1) * HG * DH]               # [D, 1024]
        woT = np.ascontiguousarray(
            Wog.T.reshape(HG, P, D).transpose(1, 0, 2))          # [128, 8, D]
        in_maps.append({
            'xT': xT, 'wqkT': wqkT, 'wvT': wvT, 'woT': woT,
            'cosT': cosT, 'sinP': sinP, 'maskT': maskT, 'onesb': onesb,
        })
    return in_maps


def kernel(x, Wqkv, Wo):
    from concourse.bass_utils import run_bass_kernel_spmd

    if 'nc' not in _CACHE:
        _CACHE['nc'] = _build_program()
    nc = _CACHE['nc']

    in_maps = _host_inputs(np.asarray(x, dtype=np.float32),
                           np.asarray(Wqkv, dtype=np.float32),
                           np.asarray(Wo, dtype=np.float32))
    res = run_bass_kernel_spmd(nc, in_maps, core_ids=list(range(8)))
    outs = [res.results[c]['out'].reshape(S, D) for c in range(8)]
    full = np.empty((B, S, D), dtype=np.float32)
    for b in range(B):
        full[b] = outs[2 * b] + outs[2 * b + 1]
    return full
